# revision 24
# baseline (speedup 1.0000x reference)
"""HEX loss kernel for Trainium2 (8 NeuronCores, batch-parallel, raw Bass).

Math: the chain junction-tree distribution factorizes into independent
Bernoullis with P(y_v=1) = sigmoid(fs[b,v]); hence
    loss = mean_b softplus(-fs[b, labels[b]])

Implementation: only fs[b, labels[b]] matters. Rows are assigned to
cores/slots on the host so that slot s = q*C + j holds a row whose
label lies in 32-column block q (8 blocks, C=640 slots each, padded
with duplicate rows). The device then needs just ONE diagonal strided
SWDGE cast-DMA that reads each slot's 32-float block (0.64 MB instead
of 4 MB per core), an additive host-built mask (+BIG on pad slots so
they contribute softplus=0, 0 at the label column, -BIG elsewhere) +
grouped reduce_max to pick the target, and softplus(-sel) =
Ln(1 + Exp(-sel)) on ACT with the free-dim sum from the activation
accumulator. The output store is not waited on (runtime quiesces DMA
queues at NEFF end; an explicit wait costs ~8 us of HBM-write receipt).
Host sums the 8x128 partials / B.
"""

import numpy as np

B = 32768
V = 256
N_CORES = 8
BL = B // N_CORES   # 4096 rows per core
P = 128
K = 16              # floats per block read per row
NQ = V // K         # 16 column blocks
C = 384             # padded slots per (core, block); 3*128
G = C // P          # 3
SLOTS = NQ * C      # 6144 slots per core
W = SLOTS * K // P  # 768 free-dim elems per partition
SEL = SLOTS // P    # 48 selected values per partition
BIG = 1.0e30

_CACHE = {}


def _build():
    from contextlib import ExitStack

    import concourse.bass as bass
    import concourse.tile as tile  # noqa
    from concourse import bacc, mybir

    f32 = mybir.dt.float32
    bf16 = mybir.dt.bfloat16
    Act = mybir.ActivationFunctionType
    Alu = mybir.AluOpType

    nc = bacc.Bacc(
        "TRN2",
        target_bir_lowering=False,
        debug=False,
        enable_asserts=False,
        num_devices=N_CORES,
    )

    fsp_d = nc.dram_tensor("fsp", [SLOTS, V], f32, kind="ExternalInput").ap()
    msk_d = nc.dram_tensor("msk", [P, W], bf16, kind="ExternalInput").ap()
    out_d = nc.dram_tensor("out", [P, 1], f32, kind="ExternalOutput").ap()

    # diagonal views, one per g: element (p, q, k) = fsp[q*C + g*P + p, K*q + k]
    # (the DMA AP balancer allows at most 3 dims, so the g axis is unrolled)
    fs_diag = [
        bass.AP(
            fsp_d.tensor,
            g * P * V,
            [[V, P], [C * V + K, NQ], [1, K]],
        )
        for g in range(G)
    ]

    with ExitStack() as ctx:
        msk = ctx.enter_context(nc.sbuf_tensor([P, W], bf16))
        gath = ctx.enter_context(nc.sbuf_tensor([P, W], f32))
        t = ctx.enter_context(nc.sbuf_tensor([P, W], bf16))
        sel = ctx.enter_context(nc.sbuf_tensor([P, SEL], f32))
        u = ctx.enter_context(nc.sbuf_tensor([P, SEL], f32))
        y = ctx.enter_context(nc.sbuf_tensor([P, SEL], f32))
        acc = ctx.enter_context(nc.sbuf_tensor([P, 1], f32))

        sem_gc = [ctx.enter_context(nc.semaphore(f"s_g{g}")) for g in range(G)]
        sem_m = ctx.enter_context(nc.semaphore("s_m"))
        sem_sel = ctx.enter_context(nc.semaphore("s_sel"))
        sem_out = ctx.enter_context(nc.semaphore("s_out"))

        blk = ctx.enter_context(nc.Block())

        # g-major column layout: chunk g owns contiguous cols [g*NQ*K, (g+1)*NQ*K)
        gview = gath.ap().rearrange("p (g q k) -> p g q k", g=G, k=K)
        CW = NQ * K  # 256 cols per chunk

        # bf16 view of the high u16 half of each f32 in gath (truncated
        # bf16) so both select operands run at bf16 DVE rates.
        gb = gath.ap().bitcast(bf16)

        def gath_hi(g):
            return bass.AP(
                gb.tensor, gb.offset + 2 * g * CW + 1, [[2 * W, P], [2, CW]]
            )

        @blk.sync
        def _(s_eng):
            s_eng.dma_start(out=msk.ap(), in_=msk_d).then_inc(sem_m, 16)
            s_eng.dma_start(out=gview[:, 0, :, :], in_=fs_diag[0]).then_inc(sem_gc[0], 16)

        @blk.gpsimd
        def _(g_eng):
            g_eng.dma_start(out=gview[:, 2, :, :], in_=fs_diag[2]).then_inc(sem_gc[2], 16)

        @blk.vector
        def _(v_eng):
            v_eng.wait_ge(sem_m, 16)
            for g in range(G):
                v_eng.wait_ge(sem_gc[g], 16)
                v_eng.tensor_add(
                    t.ap()[:, g * CW : (g + 1) * CW],
                    gath_hi(g),
                    msk.ap()[:, g * CW : (g + 1) * CW],
                )
                v_eng.drain()
                v_eng.tensor_reduce(
                    sel.ap()[:, g * NQ : (g + 1) * NQ],
                    t.ap()[:, g * CW : (g + 1) * CW].rearrange(
                        "p (c k) -> p c k", k=K
                    ),
                    axis=mybir.AxisListType.X,
                    op=Alu.max,
                )
            v_eng.drain()
            v_eng.nop().then_inc(sem_sel, 1)

        @blk.scalar
        def _(a_eng):
            from concourse.hw_specs import get_activation_tables

            tabs = list(get_activation_tables(nc.m.arch).items())
            tid = next(
                i for i, (n, s) in enumerate(tabs) if Act.Exp in s and Act.Ln in s
            )
            a_eng.add_instruction(
                mybir.InstLoadActFuncSet(
                    name=nc.get_next_instruction_name(),
                    ins=[],
                    outs=[],
                    act_func_set_id=tid,
                )
            )
            a_eng.dma_start(out=gview[:, 1, :, :], in_=fs_diag[1]).then_inc(sem_gc[1], 16)
            a_eng.wait_ge(sem_sel, 1)
            a_eng.activation(u.ap(), sel.ap(), Act.Exp, scale=-1.0)
            a_eng.drain()
            a_eng.activation(y.ap(), u.ap(), Act.Ln, bias=1.0, accum_out=acc.ap())
            a_eng.drain()
            # no wait on sem_out: the ~8us HBM-write receipt would sit on the
            # critical path; the runtime quiesces DMA queues at NEFF end.
            a_eng.dma_start(out=out_d, in_=acc.ap()).then_inc(sem_out, 16)

    nc.compile()
    return nc


def _get_nc():
    if "nc" not in _CACHE:
        _CACHE["nc"] = _build()
    return _CACHE["nc"]


def _shard_inputs(fs, labels):
    import ml_dtypes

    fs = np.ascontiguousarray(np.asarray(fs, dtype=np.float32))
    labels = np.asarray(labels).astype(np.int64)
    q_all = labels // K          # column block of each row
    kk_all = labels % K          # position within the block

    # Assign rows to (core, block-bucket) with global balancing: rows of each
    # block q are dealt round-robin across cores, so every (core, q) bucket
    # holds <= ceil(count_q / 8) <= C rows.
    order = np.argsort(q_all, kind="stable")
    counts = np.bincount(q_all, minlength=NQ)
    assert counts.max() <= C * N_CORES, counts
    rows_by_cq = [[[] for _ in range(NQ)] for _ in range(N_CORES)]
    pos = 0
    for q in range(NQ):
        rows_q = order[pos : pos + counts[q]]
        pos += counts[q]
        for i, r in enumerate(rows_q):
            rows_by_cq[i % N_CORES][q].append(r)

    in_maps = []
    for c in range(N_CORES):
        slot_rows = np.zeros(SLOTS, dtype=np.int64)
        pad = np.ones(SLOTS, dtype=bool)
        for q in range(NQ):
            rows = rows_by_cq[c][q]
            n = len(rows)
            assert n <= C, (c, q, n)
            slot_rows[q * C : q * C + n] = rows
            pad[q * C : q * C + n] = False
        fsp = fs[slot_rows]  # [SLOTS, V]

        # mask in slot layout -> [P, W]: slot s = q*C + g*P + p maps to
        # partition p, free offset q*(G*K) + g*K
        kk = kk_all[slot_rows]
        m = np.full((SLOTS, K), -BIG, dtype=np.float32)
        m[np.arange(SLOTS), kk] = 0.0
        m[pad] = BIG
        msk = (
            m.reshape(NQ, G, P, K)
            .transpose(2, 1, 0, 3)
            .reshape(P, W)
            .astype(ml_dtypes.bfloat16)
        )
        in_maps.append(
            {"fsp": fsp, "msk": np.ascontiguousarray(msk)}
        )
    return in_maps


def kernel(fs, labels, _trace=False, _trace_kwargs=None):
    from concourse.bass_utils import run_bass_kernel_spmd

    nc = _get_nc()
    in_maps = _shard_inputs(fs, labels)
    res = run_bass_kernel_spmd(
        nc,
        in_maps,
        core_ids=list(range(N_CORES)),
        trace=_trace,
        **(_trace_kwargs or {}),
    )
    total = np.float64(0.0)
    for c in range(N_CORES):
        total += res.results[c]["out"].astype(np.float64).sum()
    loss = total / np.float64(B)
    if _trace:
        return np.float64(loss), res
    return np.asarray(loss, dtype=np.float64)


# revision 28
# speedup vs baseline: 1.0691x; 1.0691x over previous
"""HEX loss kernel for Trainium2 (8 NeuronCores, batch-parallel, raw Bass).

Math: the chain junction-tree distribution factorizes into independent
Bernoullis with P(y_v=1) = sigmoid(fs[b,v]); hence
    loss = mean_b softplus(-fs[b, labels[b]])

Implementation: only fs[b, labels[b]] matters. Rows are assigned to
cores/slots on the host so that slot s = q*C + j holds a row whose
label lies in 32-column block q (8 blocks, C=640 slots each, padded
with duplicate rows). The device then needs just ONE diagonal strided
SWDGE cast-DMA that reads each slot's 32-float block (0.64 MB instead
of 4 MB per core), an additive host-built mask (+BIG on pad slots so
they contribute softplus=0, 0 at the label column, -BIG elsewhere) +
grouped reduce_max to pick the target, and softplus(-sel) =
Ln(1 + Exp(-sel)) on ACT with the free-dim sum from the activation
accumulator. The output store is not waited on (runtime quiesces DMA
queues at NEFF end; an explicit wait costs ~8 us of HBM-write receipt).
Host sums the 8x128 partials / B.
"""

import numpy as np

B = 32768
V = 256
N_CORES = 8
BL = B // N_CORES   # 4096 rows per core
P = 128
K = 16              # floats per block read per row
NQ = V // K         # 16 column blocks
C = 384             # padded slots per (core, block); 3*128
G = C // P          # 3
SLOTS = NQ * C      # 6144 slots per core
W = SLOTS * K // P  # 768 free-dim elems per partition
SEL = SLOTS // P    # 48 selected values per partition
BIG = 1.0e30

_CACHE = {}


def _build():
    from contextlib import ExitStack

    import concourse.bass as bass
    import concourse.tile as tile  # noqa
    from concourse import bacc, mybir

    f32 = mybir.dt.float32
    bf16 = mybir.dt.bfloat16
    Act = mybir.ActivationFunctionType
    Alu = mybir.AluOpType

    nc = bacc.Bacc(
        "TRN2",
        target_bir_lowering=False,
        debug=False,
        enable_asserts=False,
        num_devices=N_CORES,
    )

    fsp_d = nc.dram_tensor("fsp", [SLOTS, V], f32, kind="ExternalInput").ap()
    msk_d = nc.dram_tensor("msk", [P, W], bf16, kind="ExternalInput").ap()
    out_d = nc.dram_tensor("out", [P, 1], f32, kind="ExternalOutput").ap()

    # diagonal views, one per g: element (p, q, k) = fsp[q*C + g*P + p, K*q + k]
    # (the DMA AP balancer allows at most 3 dims, so the g axis is unrolled)
    fs_diag = [
        bass.AP(
            fsp_d.tensor,
            g * P * V,
            [[V, P], [C * V + K, NQ], [1, K]],
        )
        for g in range(G)
    ]

    with ExitStack() as ctx:
        msk = ctx.enter_context(nc.sbuf_tensor([P, W], bf16))
        gath = ctx.enter_context(nc.sbuf_tensor([P, W], f32))
        t = ctx.enter_context(nc.sbuf_tensor([P, W], bf16))
        sel = ctx.enter_context(nc.sbuf_tensor([P, SEL], f32))
        u = ctx.enter_context(nc.sbuf_tensor([P, SEL], f32))
        y = ctx.enter_context(nc.sbuf_tensor([P, SEL], f32))
        acc = ctx.enter_context(nc.sbuf_tensor([P, 1], f32))

        sem_gc = [ctx.enter_context(nc.semaphore(f"s_g{g}")) for g in range(G)]
        sem_mc = [ctx.enter_context(nc.semaphore(f"s_m{g}")) for g in range(G)]
        sem_sel = ctx.enter_context(nc.semaphore("s_sel"))
        sem_out = ctx.enter_context(nc.semaphore("s_out"))

        blk = ctx.enter_context(nc.Block())

        # g-major column layout: chunk g owns contiguous cols [g*NQ*K, (g+1)*NQ*K)
        gview = gath.ap().rearrange("p (g q k) -> p g q k", g=G, k=K)
        CW = NQ * K  # 256 cols per chunk

        # bf16 view of the high u16 half of each f32 in gath (truncated
        # bf16) so both select operands run at bf16 DVE rates.
        gb = gath.ap().bitcast(bf16)

        def gath_hi(g):
            return bass.AP(
                gb.tensor, gb.offset + 2 * g * CW + 1, [[2 * W, P], [2, CW]]
            )

        @blk.sync
        def _(s_eng):
            s_eng.dma_start(out=msk.ap(), in_=msk_d).then_inc(sem_mc[0], 16)
            s_eng.dma_start(out=gview[:, 0, :, :], in_=fs_diag[0]).then_inc(sem_gc[0], 16)

        @blk.vector
        def _(v_eng):
            v_eng.wait_ge(sem_mc[0], 16)
            for g in range(G):
                v_eng.wait_ge(sem_gc[g], 16)
                v_eng.tensor_add(
                    t.ap()[:, g * CW : (g + 1) * CW],
                    gath_hi(g),
                    msk.ap()[:, g * CW : (g + 1) * CW],
                )
                v_eng.drain()
                v_eng.tensor_reduce(
                    sel.ap()[:, g * NQ : (g + 1) * NQ],
                    t.ap()[:, g * CW : (g + 1) * CW].rearrange(
                        "p (c k) -> p c k", k=K
                    ),
                    axis=mybir.AxisListType.X,
                    op=Alu.max,
                )
            v_eng.drain()
            v_eng.nop().then_inc(sem_sel, 1)

        @blk.scalar
        def _(a_eng):
            from concourse.hw_specs import get_activation_tables

            tabs = list(get_activation_tables(nc.m.arch).items())
            tid = next(
                i for i, (n, s) in enumerate(tabs) if Act.Exp in s and Act.Ln in s
            )
            a_eng.add_instruction(
                mybir.InstLoadActFuncSet(
                    name=nc.get_next_instruction_name(),
                    ins=[],
                    outs=[],
                    act_func_set_id=tid,
                )
            )
            a_eng.dma_start(out=gview[:, 1, :, :], in_=fs_diag[1]).then_inc(sem_gc[1], 16)
            a_eng.dma_start(out=gview[:, 2, :, :], in_=fs_diag[2]).then_inc(sem_gc[2], 16)
            a_eng.wait_ge(sem_sel, 1)
            a_eng.activation(u.ap(), sel.ap(), Act.Exp, scale=-1.0)
            a_eng.drain()
            a_eng.activation(y.ap(), u.ap(), Act.Ln, bias=1.0, accum_out=acc.ap())
            a_eng.drain()
            # no wait on sem_out: the ~8us HBM-write receipt would sit on the
            # critical path; the runtime quiesces DMA queues at NEFF end.
            a_eng.dma_start(out=out_d, in_=acc.ap()).then_inc(sem_out, 16)

    nc.compile()
    return nc


def _get_nc():
    if "nc" not in _CACHE:
        _CACHE["nc"] = _build()
    return _CACHE["nc"]


def _shard_inputs(fs, labels):
    import ml_dtypes

    fs = np.ascontiguousarray(np.asarray(fs, dtype=np.float32))
    labels = np.asarray(labels).astype(np.int64)
    q_all = labels // K          # column block of each row
    kk_all = labels % K          # position within the block

    # Assign rows to (core, block-bucket) with global balancing: rows of each
    # block q are dealt round-robin across cores, so every (core, q) bucket
    # holds <= ceil(count_q / 8) <= C rows.
    order = np.argsort(q_all, kind="stable")
    counts = np.bincount(q_all, minlength=NQ)
    assert counts.max() <= C * N_CORES, counts
    rows_by_cq = [[[] for _ in range(NQ)] for _ in range(N_CORES)]
    pos = 0
    for q in range(NQ):
        rows_q = order[pos : pos + counts[q]]
        pos += counts[q]
        for i, r in enumerate(rows_q):
            rows_by_cq[i % N_CORES][q].append(r)

    in_maps = []
    for c in range(N_CORES):
        slot_rows = np.zeros(SLOTS, dtype=np.int64)
        pad = np.ones(SLOTS, dtype=bool)
        for q in range(NQ):
            rows = rows_by_cq[c][q]
            n = len(rows)
            assert n <= C, (c, q, n)
            slot_rows[q * C : q * C + n] = rows
            pad[q * C : q * C + n] = False
        fsp = fs[slot_rows]  # [SLOTS, V]

        # mask in slot layout -> [P, W]: slot s = q*C + g*P + p maps to
        # partition p, free offset q*(G*K) + g*K
        kk = kk_all[slot_rows]
        m = np.full((SLOTS, K), -BIG, dtype=np.float32)
        m[np.arange(SLOTS), kk] = 0.0
        m[pad] = BIG
        msk = (
            m.reshape(NQ, G, P, K)
            .transpose(2, 1, 0, 3)
            .reshape(P, W)
            .astype(ml_dtypes.bfloat16)
        )
        in_maps.append(
            {"fsp": fsp, "msk": np.ascontiguousarray(msk)}
        )
    return in_maps


def kernel(fs, labels, _trace=False, _trace_kwargs=None):
    from concourse.bass_utils import run_bass_kernel_spmd

    nc = _get_nc()
    in_maps = _shard_inputs(fs, labels)
    res = run_bass_kernel_spmd(
        nc,
        in_maps,
        core_ids=list(range(N_CORES)),
        trace=_trace,
        **(_trace_kwargs or {}),
    )
    total = np.float64(0.0)
    for c in range(N_CORES):
        total += res.results[c]["out"].astype(np.float64).sum()
    loss = total / np.float64(B)
    if _trace:
        return np.float64(loss), res
    return np.asarray(loss, dtype=np.float64)


# revision 31
# speedup vs baseline: 1.2198x; 1.1410x over previous
"""HEX loss kernel for Trainium2 (8 NeuronCores, batch-parallel, raw Bass).

Math: the chain junction-tree distribution factorizes into independent
Bernoullis with P(y_v=1) = sigmoid(fs[b,v]); hence
    loss = mean_b softplus(-fs[b, labels[b]])

Implementation: only fs[b, labels[b]] matters. Rows are assigned to
cores/slots on the host so that slot s = q*C + j holds a row whose
label lies in 32-column block q (8 blocks, C=640 slots each, padded
with duplicate rows). The device then needs just ONE diagonal strided
SWDGE cast-DMA that reads each slot's 32-float block (0.64 MB instead
of 4 MB per core), an additive host-built mask (+BIG on pad slots so
they contribute softplus=0, 0 at the label column, -BIG elsewhere) +
grouped reduce_max to pick the target, and softplus(-sel) =
Ln(1 + Exp(-sel)) on ACT with the free-dim sum from the activation
accumulator. The output store is not waited on (runtime quiesces DMA
queues at NEFF end; an explicit wait costs ~8 us of HBM-write receipt).
Host sums the 8x128 partials / B.
"""

import numpy as np

B = 32768
V = 256
N_CORES = 8
BL = B // N_CORES   # 4096 rows per core
P = 128
K = 16              # floats per block read per row
NQ = V // K         # 16 column blocks
C = 384             # padded slots per (core, block); 3*128
G = C // P          # 3
SLOTS = NQ * C      # 6144 slots per core
W = SLOTS * K // P  # 768 free-dim elems per partition
SEL = SLOTS // P    # 48 selected values per partition
BIG = 1.0e30

_CACHE = {}


def _build():
    from contextlib import ExitStack

    import concourse.bass as bass
    import concourse.tile as tile  # noqa
    from concourse import bacc, mybir

    f32 = mybir.dt.float32
    bf16 = mybir.dt.bfloat16
    Act = mybir.ActivationFunctionType
    Alu = mybir.AluOpType

    nc = bacc.Bacc(
        "TRN2",
        target_bir_lowering=False,
        debug=False,
        enable_asserts=False,
        num_devices=N_CORES,
    )

    # fsb[q, s, k] = fs_perm[s, q*K + k]: block-major staging so each
    # bucket's block sits contiguous in DRAM (contiguous M2S reads).
    fsb_d = nc.dram_tensor("fsb", [NQ, SLOTS, K], f32, kind="ExternalInput").ap()
    msk_d = nc.dram_tensor("msk", [P, W], bf16, kind="ExternalInput").ap()
    out_d = nc.dram_tensor("out", [P, 1], f32, kind="ExternalOutput").ap()

    # diagonal views, one per g: element (p, q, k) = fsb[q, q*C + g*P + p, k]
    # (the DMA AP balancer allows at most 3 dims, so the g axis is unrolled)
    fs_diag = [
        bass.AP(
            fsb_d.tensor,
            g * P * K,
            [[K, P], [(SLOTS + C) * K, NQ], [1, K]],
        )
        for g in range(G)
    ]

    with ExitStack() as ctx:
        msk = ctx.enter_context(nc.sbuf_tensor([P, W], bf16))
        gath = ctx.enter_context(nc.sbuf_tensor([P, W], f32))
        t = ctx.enter_context(nc.sbuf_tensor([P, W], bf16))
        sel = ctx.enter_context(nc.sbuf_tensor([P, SEL], f32))
        u = ctx.enter_context(nc.sbuf_tensor([P, SEL], f32))
        y = ctx.enter_context(nc.sbuf_tensor([P, SEL], f32))
        acc = ctx.enter_context(nc.sbuf_tensor([P, 1], f32))

        sem_gc = [ctx.enter_context(nc.semaphore(f"s_g{g}")) for g in range(G)]
        sem_mc = [ctx.enter_context(nc.semaphore(f"s_m{g}")) for g in range(G)]
        sem_sel = ctx.enter_context(nc.semaphore("s_sel"))
        sem_out = ctx.enter_context(nc.semaphore("s_out"))

        blk = ctx.enter_context(nc.Block())

        # g-major column layout: chunk g owns contiguous cols [g*NQ*K, (g+1)*NQ*K)
        gview = gath.ap().rearrange("p (g q k) -> p g q k", g=G, k=K)
        CW = NQ * K  # 256 cols per chunk

        # bf16 view of the high u16 half of each f32 in gath (truncated
        # bf16) so both select operands run at bf16 DVE rates.
        gb = gath.ap().bitcast(bf16)

        def gath_hi(g):
            return bass.AP(
                gb.tensor, gb.offset + 2 * g * CW + 1, [[2 * W, P], [2, CW]]
            )

        @blk.sync
        def _(s_eng):
            s_eng.dma_start(out=msk.ap(), in_=msk_d).then_inc(sem_mc[0], 16)
            s_eng.dma_start(out=gview[:, 0, :, :], in_=fs_diag[0]).then_inc(sem_gc[0], 16)

        @blk.vector
        def _(v_eng):
            v_eng.wait_ge(sem_mc[0], 16)
            for g in range(G):
                v_eng.wait_ge(sem_gc[g], 16)
                v_eng.tensor_add(
                    t.ap()[:, g * CW : (g + 1) * CW],
                    gath_hi(g),
                    msk.ap()[:, g * CW : (g + 1) * CW],
                )
                v_eng.drain()
                v_eng.tensor_reduce(
                    sel.ap()[:, g * NQ : (g + 1) * NQ],
                    t.ap()[:, g * CW : (g + 1) * CW].rearrange(
                        "p (c k) -> p c k", k=K
                    ),
                    axis=mybir.AxisListType.X,
                    op=Alu.max,
                )
            v_eng.drain()
            v_eng.nop().then_inc(sem_sel, 1)

        @blk.scalar
        def _(a_eng):
            from concourse.hw_specs import get_activation_tables

            tabs = list(get_activation_tables(nc.m.arch).items())
            tid = next(
                i for i, (n, s) in enumerate(tabs) if Act.Exp in s and Act.Ln in s
            )
            a_eng.add_instruction(
                mybir.InstLoadActFuncSet(
                    name=nc.get_next_instruction_name(),
                    ins=[],
                    outs=[],
                    act_func_set_id=tid,
                )
            )
            a_eng.dma_start(out=gview[:, 1, :, :], in_=fs_diag[1]).then_inc(sem_gc[1], 16)
            a_eng.dma_start(out=gview[:, 2, :, :], in_=fs_diag[2]).then_inc(sem_gc[2], 16)
            a_eng.wait_ge(sem_sel, 1)
            a_eng.activation(u.ap(), sel.ap(), Act.Exp, scale=-1.0)
            a_eng.drain()
            a_eng.activation(y.ap(), u.ap(), Act.Ln, bias=1.0, accum_out=acc.ap())
            a_eng.drain()
            # no wait on sem_out: the ~8us HBM-write receipt would sit on the
            # critical path; the runtime quiesces DMA queues at NEFF end.
            a_eng.dma_start(out=out_d, in_=acc.ap()).then_inc(sem_out, 16)

    nc.compile()
    return nc


def _get_nc():
    if "nc" not in _CACHE:
        _CACHE["nc"] = _build()
    return _CACHE["nc"]


def _shard_inputs(fs, labels):
    import ml_dtypes

    fs = np.ascontiguousarray(np.asarray(fs, dtype=np.float32))
    labels = np.asarray(labels).astype(np.int64)
    q_all = labels // K          # column block of each row
    kk_all = labels % K          # position within the block

    # Assign rows to (core, block-bucket) with global balancing: rows of each
    # block q are dealt round-robin across cores, so every (core, q) bucket
    # holds <= ceil(count_q / 8) <= C rows.
    order = np.argsort(q_all, kind="stable")
    counts = np.bincount(q_all, minlength=NQ)
    assert counts.max() <= C * N_CORES, counts
    rows_by_cq = [[[] for _ in range(NQ)] for _ in range(N_CORES)]
    pos = 0
    for q in range(NQ):
        rows_q = order[pos : pos + counts[q]]
        pos += counts[q]
        for i, r in enumerate(rows_q):
            rows_by_cq[i % N_CORES][q].append(r)

    in_maps = []
    for c in range(N_CORES):
        slot_rows = np.zeros(SLOTS, dtype=np.int64)
        pad = np.ones(SLOTS, dtype=bool)
        for q in range(NQ):
            rows = rows_by_cq[c][q]
            n = len(rows)
            assert n <= C, (c, q, n)
            slot_rows[q * C : q * C + n] = rows
            pad[q * C : q * C + n] = False
        fsp = fs[slot_rows]  # [SLOTS, V]
        fsb = np.ascontiguousarray(
            fsp.reshape(SLOTS, NQ, K).transpose(1, 0, 2)
        )  # [NQ, SLOTS, K]

        # mask in slot layout -> [P, W]: slot s = q*C + g*P + p maps to
        # partition p, free offset q*(G*K) + g*K
        kk = kk_all[slot_rows]
        m = np.full((SLOTS, K), -BIG, dtype=np.float32)
        m[np.arange(SLOTS), kk] = 0.0
        m[pad] = BIG
        msk = (
            m.reshape(NQ, G, P, K)
            .transpose(2, 1, 0, 3)
            .reshape(P, W)
            .astype(ml_dtypes.bfloat16)
        )
        in_maps.append(
            {"fsb": fsb, "msk": np.ascontiguousarray(msk)}
        )
    return in_maps


def kernel(fs, labels, _trace=False, _trace_kwargs=None):
    from concourse.bass_utils import run_bass_kernel_spmd

    nc = _get_nc()
    in_maps = _shard_inputs(fs, labels)
    res = run_bass_kernel_spmd(
        nc,
        in_maps,
        core_ids=list(range(N_CORES)),
        trace=_trace,
        **(_trace_kwargs or {}),
    )
    total = np.float64(0.0)
    for c in range(N_CORES):
        total += res.results[c]["out"].astype(np.float64).sum()
    loss = total / np.float64(B)
    if _trace:
        return np.float64(loss), res
    return np.asarray(loss, dtype=np.float64)
